# revision 1
# baseline (speedup 1.0000x reference)
"""Trainium2 Bass kernel for capped-softmax multi-head attention.

Module: x -> qkv -> q/k LayerNorm -> scores -> tanh-cap softmax -> AV -> proj

Sharding over 8 NeuronCores: core c = b*4 + g handles batch b (of 2) and
head group g (4 of the 16 heads).  Data-parallel on batch, tensor-parallel
on heads; proj is row-parallel with the 4 partial (1024, 2048) outputs per
batch summed on the host (+ proj_b).

Per-core pipeline (bf16 matmul operands, fp32 accumulate):
  phase 1: QKV token-major (bias via K=1 ones matmul), LayerNorm stats +
           apply over head_dim on the q/k slices, PE-transpose of q/k
           head-pairs to d-major (scale folded into k when biases are 0),
           v copied into token-major v-hat tiles with a ones column.
  phase 2: per (pair, 512-query chunk, key tile): scores for both heads into
           one 2-bank PSUM tile, tanh-cap via a fused custom DVE op
           (degree-5 odd minimax polynomial, in-place in PSUM), exp on
           ScalarE (PSUM -> SBUF bf16), AV with lhsT = v-hat so row 64 of
           the feat-major accumulator is the softmax denominator; divide
           via approx-reciprocal + gpsimd partition-broadcast + one
           tensor_tensor multiply.
  phase 3: proj feat-major out[of, t] = sum_f Wp[f, of] y[f, t], overlapped
           with phase 2 (shared pool scope; per-chunk yfm tiles).

The softmax skips max-subtraction: capped scores are in [-9.7, 9.7] so exp
cannot overflow, and softmax is shift-invariant.
"""

import numpy as np

import concourse.bass as bass
import concourse.bacc as bacc
import concourse.tile as tile
from concourse import mybir
from concourse.bass_utils import run_bass_kernel_spmd
from concourse.masks import make_identity

F32 = mybir.dt.float32
BF16 = mybir.dt.bfloat16
MMDT = BF16          # dtype for matmul operands

B, N, C = 2, 2048, 1024
H, D = 16, 64
G = 4              # heads per core
NCORES = 8
EPS = 1e-5

# degree-5 odd minimax fit of 30*tanh(s/30) on [-10, 10]; max abs err 1.8e-5
CAP_A = -3.700163173e-04
CAP_B = 1.541633493e-07
CAP_CLAMP = 10.0

TT = N // 128      # 16 token tiles
KI = C // 128      # 8 contraction chunks for qkv
ICN = N // 512     # 4 query chunks
JTN = N // 128     # 16 key tiles

_CAP_OP = None


def _register_cap_op():
    """Register the fused clamp+odd-poly tanh-cap custom DVE op."""
    global _CAP_OP
    if _CAP_OP is not None:
        return _CAP_OP
    import concourse.dve_ops as dve_ops
    from concourse.dve_spec import Spec, Src0, C0, C1, C2, Zero, One, sq, maxx, minn, lower
    from concourse.dve_uop import DveOpSpec

    name = "TANH_CAP_ANT"
    for op in dve_ops.OPS:
        if op.name == name:
            _CAP_OP = op
            return op

    xc = maxx(minn(Src0, C2), Zero - C2)
    t = sq(xc)
    body = xc * (One + t * (C0 + t * C1))

    def ref(in0, in1, s0, s1, imm2):
        z = np.clip(in0, -imm2, imm2).astype(np.float32)
        tt = (z * z).astype(np.float32)
        return (z * (np.float32(1.0) + tt * (np.float32(s0) + tt * np.float32(s1)))).astype(np.float32)

    spec = Spec(body=body, reference=ref)
    opcode = dve_ops._CUSTOM_DVE_ROW_BASE + len(dve_ops.OPS)
    shas = {}
    for ver in ("v3", "v4"):
        ds = DveOpSpec(name=name, opcode=opcode, uops=lower(spec, ver=ver),
                       rd1_en=False)
        shas[ver] = ds.sha(ver)
    cap = dve_ops.DveOp(name, spec, subdim=False, uops_sha=shas)
    dve_ops.OPS.append(cap)
    dve_ops.CUSTOM_DVE_SPECS[name] = spec
    dve_ops._SUB_OPCODE_FOR_NAME[name] = opcode
    _CAP_OP = cap
    return cap


def _build_nc(dve_cap_frac=1.0, debug=False, phases=3, rep2=1, rep_all=1,
              fold_scales=True):
    """Trace the single-core Tile kernel (same program for all 8 cores)."""
    cap_op = _register_cap_op()
    nc = bacc.Bacc(trn_type="TRN2")

    xt = nc.dram_tensor("xt", [128, TT, KI, 128], MMDT, kind="ExternalInput")
    wqkv = nc.dram_tensor("wqkv_t", [C, 3 * G * D], MMDT, kind="ExternalInput")
    bqkv = nc.dram_tensor("bqkv", [1, 3 * G * D], MMDT, kind="ExternalInput")
    wproj = nc.dram_tensor("wproj_t", [G * D, C], MMDT, kind="ExternalInput")
    lnq_s = nc.dram_tensor("lnq_s", [128, 1], F32, kind="ExternalInput")
    lnq_b = nc.dram_tensor("lnq_b", [128, 1], F32, kind="ExternalInput")
    lnk_s = nc.dram_tensor("lnk_s", [128, 1], F32, kind="ExternalInput")
    lnk_b = nc.dram_tensor("lnk_b", [128, 1], F32, kind="ExternalInput")
    out_fm = nc.dram_tensor("out_fm", [C, N], F32, kind="ExternalOutput")

    with tile.TileContext(nc) as tc:
        with tc.tile_pool(name="singles", bufs=1) as singles:
            ident = singles.tile([128, 128], MMDT)
            make_identity(nc, ident)
            ones1 = singles.tile([1, 128], MMDT)
            nc.vector.memset(ones1, 1.0)
            eps_t = singles.tile([128, 1], F32)
            nc.vector.memset(eps_t, EPS)

            w_sb = singles.tile([128, KI, 3 * G * D], MMDT)
            nc.sync.dma_start(out=w_sb,
                              in_=wqkv[:, :].rearrange("(ki p) f -> p ki f", p=128))
            bq_sb = singles.tile([1, 3 * G * D], MMDT)
            nc.sync.dma_start(out=bq_sb, in_=bqkv[:, :])
            wp_sb = singles.tile([128, 2, C], MMDT)
            nc.sync.dma_start(out=wp_sb,
                              in_=wproj[:, :].rearrange("(fc p) f -> p fc f", p=128))
            ln_sb = {}
            for nm, t_ in (("qs", lnq_s), ("qb", lnq_b), ("ks", lnk_s), ("kb", lnk_b)):
                s = singles.tile([128, 1], F32, name=f"ln_{nm}")
                nc.sync.dma_start(out=s, in_=t_[:, :])
                ln_sb[nm] = s

            # persistent big SBUF tensors
            qdm = singles.tile([128, 2, N], MMDT)   # q d-major, head pairs
            kdm = singles.tile([128, 2, N], MMDT)   # k d-major, head pairs
            vhat = singles.tile([128, G, JTN, 65], MMDT)  # v token-major + ones
            nc.vector.memset(vhat, 1.0)             # col 64 stays 1.0
            yfm = [[singles.tile([128, 512], MMDT, name=f"yfm_{pp}_{cc}")
                    for cc in range(ICN)] for pp in range(2)]

            # ---------------- phase 1: QKV + LN + transposes ----------------
            with tc.tile_pool(name="p1sb", bufs=6) as p1sb, \
                 tc.tile_pool(name="p1st", bufs=8) as p1st, \
                 tc.tile_pool(name="xtp", bufs=12) as xtp, \
                 tc.tile_pool(name="p1ps", bufs=5, space="PSUM") as p1ps, \
                 tc.tile_pool(name="p1psv", bufs=2, space="PSUM") as p1psv, \
                 tc.tile_pool(name="p1pst", bufs=1, space="PSUM") as p1pst:
                for tt_i in range(TT * rep_all):
                    tt_i = tt_i % TT
                    tsl = slice(tt_i * 128, (tt_i + 1) * 128)
                    xt_t = xtp.tile([128, KI, 128], MMDT, name="xt_t")
                    nc.sync.dma_start(out=xt_t, in_=xt[:, tt_i, :, :])
                    xts = [xt_t[:, ki, :] for ki in range(KI)]
                    p0 = p1ps.tile([128, 512], F32)      # q(256) | k(256)
                    p1 = p1psv.tile([128, 256], F32)     # v(256)
                    for ki in range(KI):
                        nc.tensor.matmul(p0, xts[ki], w_sb[:, ki, 0:512],
                                         start=(ki == 0), stop=False)
                    nc.tensor.matmul(p0, ones1, bq_sb[:, 0:512],
                                     start=False, stop=True)
                    for ki in range(KI):
                        nc.tensor.matmul(p1, xts[ki], w_sb[:, ki, 512:768],
                                         start=(ki == 0), stop=False)
                    nc.tensor.matmul(p1, ones1, bq_sb[:, 512:768],
                                     start=False, stop=True)

                    # LayerNorm stats for the 8 (q,k)-head groups of 64
                    sums = p1st.tile([128, 8], F32)
                    nc.vector.tensor_reduce(sums, p0.rearrange("p (g d) -> p g d", g=8),
                                            axis=mybir.AxisListType.X,
                                            op=mybir.AluOpType.add)
                    sq_t = p1sb.tile([128, 512], F32, name="sq_t")
                    nc.scalar.activation(sq_t, p0, mybir.ActivationFunctionType.Square)
                    sqs = p1st.tile([128, 8], F32)
                    nc.vector.tensor_reduce(sqs, sq_t.rearrange("p (g d) -> p g d", g=8),
                                            axis=mybir.AxisListType.X,
                                            op=mybir.AluOpType.add)
                    mean = p1st.tile([128, 8], F32)
                    nc.scalar.mul(mean, sums, 1.0 / 64)
                    msq = p1st.tile([128, 8], F32)
                    nc.scalar.mul(msq, sqs, 1.0 / 64)
                    var = p1st.tile([128, 8], F32)
                    nc.vector.tensor_mul(var, mean, mean)
                    nc.vector.tensor_sub(var, msq, var)
                    std = p1st.tile([128, 8], F32)
                    nc.scalar.activation(std, var, mybir.ActivationFunctionType.Sqrt,
                                         bias=eps_t)
                    rstd = p1st.tile([128, 8], F32)
                    nc.vector.reciprocal(rstd, std)

                    qk = p1sb.tile([128, 512], MMDT, name="qk")
                    for gi in range(8):
                        nc.vector.tensor_scalar(
                            out=qk[:, gi * 64:(gi + 1) * 64],
                            in0=p0[:, gi * 64:(gi + 1) * 64],
                            scalar1=mean[:, gi:gi + 1],
                            scalar2=rstd[:, gi:gi + 1],
                            op0=mybir.AluOpType.subtract,
                            op1=mybir.AluOpType.mult,
                        )

                    # transpose q/k head-pairs to d-major (+ scale/bias)
                    for pi in range(2):
                        for is_k, dm, s_ap, b_ap in ((0, qdm, ln_sb["qs"], ln_sb["qb"]),
                                                     (1, kdm, ln_sb["ks"], ln_sb["kb"])):
                            tp = p1pst.tile([128, 128], MMDT, name="tp")
                            src = qk[:, is_k * 256 + pi * 128:
                                     is_k * 256 + (pi + 1) * 128]
                            nc.tensor.transpose(tp, src, ident)
                            if fold_scales and not is_k:
                                nc.scalar.copy(dm[:, pi, tsl], tp)
                            elif fold_scales:
                                nc.scalar.activation(
                                    dm[:, pi, tsl], tp,
                                    mybir.ActivationFunctionType.Copy,
                                    scale=s_ap)
                            else:
                                nc.vector.tensor_scalar(
                                    out=dm[:, pi, tsl], in0=tp,
                                    scalar1=s_ap, scalar2=b_ap,
                                    op0=mybir.AluOpType.mult,
                                    op1=mybir.AluOpType.add,
                                )

                    # v -> vhat[:, :, tt_i, 0:64]
                    nc.scalar.copy(
                        out=vhat[:, :, tt_i, 0:64],
                        in_=p1.rearrange("p (g d) -> p g d", g=G),
                    )

            # ------------- phase 2 + 3: attention and projection -------------
            n_cap = 0
            with tc.tile_pool(name="sps", bufs=3, space="PSUM") as sps, \
                 tc.tile_pool(name="ops_", bufs=2, space="PSUM") as ops_, \
                 tc.tile_pool(name="esb", bufs=8) as esb, \
                 tc.tile_pool(name="rsb", bufs=6) as rsb:
                for p in range(2 * rep2 * rep_all if phases >= 2 else 0):
                    p = p % 2
                    for ic in range(ICN):
                        isl = slice(ic * 512, (ic + 1) * 512)
                        o_ps = [ops_.tile([65, 512], F32, name="o_ps")
                                for _ in range(2)]
                        for jt in range(JTN):
                            jsl = slice(jt * 128, (jt + 1) * 128)
                            # both heads' scores in one 2-bank tile [h0 | h1]
                            s_ps = sps.tile([128, 2, 512], F32, name="s_ps")
                            for hh in range(2):
                                nc.tensor.matmul(s_ps[:, hh, :],
                                                 kdm[hh * 64:(hh + 1) * 64, p, jsl],
                                                 qdm[hh * 64:(hh + 1) * 64, p, isl],
                                                 start=True, stop=True)
                            use_dve = (n_cap % 16) < round(dve_cap_frac * 16)
                            n_cap += 1
                            e_t = esb.tile([128, 2, 512], MMDT, name="e_t")
                            if use_dve:
                                nc.vector._custom_dve(cap_op, out=s_ps, in0=s_ps,
                                                      s0=CAP_A, s1=CAP_B,
                                                      imm2=CAP_CLAMP)
                                nc.scalar.activation(e_t, s_ps,
                                                     mybir.ActivationFunctionType.Exp)
                            else:
                                nc.scalar.activation(s_ps, s_ps,
                                                     mybir.ActivationFunctionType.Tanh,
                                                     scale=1.0 / 30.0)
                                nc.scalar.activation(e_t, s_ps,
                                                     mybir.ActivationFunctionType.Exp,
                                                     scale=30.0)
                            for hh in range(2):
                                nc.tensor.matmul(o_ps[hh],
                                                 vhat[:, 2 * p + hh, jt, :],
                                                 e_t[:, hh, :],
                                                 start=(jt == 0),
                                                 stop=(jt == JTN - 1))
                        for hh in range(2):
                            rs0 = rsb.tile([1, 512], F32, name="rs0")
                            nc.scalar.copy(rs0, o_ps[hh][64:65, :])
                            rs = rsb.tile([1, 512], F32, name="rs")
                            nc.vector.reciprocal_approx_fast(rs, rs0)
                            rb = rsb.tile([64, 512], F32, name="rb")
                            nc.gpsimd.partition_broadcast(rb, rs, channels=64)
                            nc.vector.tensor_mul(
                                yfm[p][ic][hh * 64:(hh + 1) * 64, :],
                                o_ps[hh][0:64, :], rb)

            # ---------------- phase 3: proj ----------------
            with tc.tile_pool(name="prps", bufs=4, space="PSUM") as prps, \
                 tc.tile_pool(name="osb2", bufs=4) as osb2:
                for ot in range(8 * rep_all if phases >= 3 else 0):
                    ot = ot % 8
                    for tci in range(ICN):
                        tsl = slice(tci * 512, (tci + 1) * 512)
                        pr = prps.tile([128, 512], F32, name="pr")
                        for fc in range(2):
                            nc.tensor.matmul(pr,
                                             wp_sb[:, fc, ot * 128:(ot + 1) * 128],
                                             yfm[fc][tci],
                                             start=(fc == 0), stop=(fc == 1))
                        ob = osb2.tile([128, 512], F32, name="ob")
                        if (ot * ICN + tci) % 2 == 0:
                            nc.scalar.copy(out=ob, in_=pr)
                        else:
                            nc.vector.tensor_copy(ob, pr)
                        nc.sync.dma_start(out=out_fm[ot * 128:(ot + 1) * 128, tsl],
                                          in_=ob)
    nc.finalize()
    return nc


_NC_CACHE = {}


def _get_nc(dve_cap_frac=1.0, debug=False, phases=3, rep2=1, rep_all=1,
            fold_scales=True):
    key = (dve_cap_frac, debug, phases, rep2, rep_all, fold_scales)
    if key not in _NC_CACHE:
        _NC_CACHE[key] = _build_nc(dve_cap_frac, debug, phases, rep2, rep_all,
                                   fold_scales)
    return _NC_CACHE[key]


def _make_in_maps(x, qkv_w, qkv_b, qn_w, qn_b, kn_w, kn_b, proj_w):
    """Returns (in_maps, fold_scales)."""
    import ml_dtypes
    mmnp = ml_dtypes.bfloat16
    x = np.asarray(x, np.float32)
    qkv_w = np.asarray(qkv_w, np.float32)
    qkv_b = np.asarray(qkv_b, np.float32)
    proj_w = np.asarray(proj_w, np.float32)
    qn_w = np.asarray(qn_w, np.float32); qn_b = np.asarray(qn_b, np.float32)
    kn_w = np.asarray(kn_w, np.float32); kn_b = np.asarray(kn_b, np.float32)

    scale = np.float32(D ** -0.5)
    fold = bool(np.all(qn_b == 0) and np.all(kn_b == 0))
    if fold:
        lnq_s = np.ones((128, 1), np.float32)
        lnq_b = np.zeros((128, 1), np.float32)
        lnk_s = (np.tile(kn_w * qn_w, 2) * scale).reshape(128, 1).astype(np.float32)
        lnk_b = np.zeros((128, 1), np.float32)
    else:
        lnq_s = (np.tile(qn_w, 2) * scale).reshape(128, 1).astype(np.float32)
        lnq_b = (np.tile(qn_b, 2) * scale).reshape(128, 1).astype(np.float32)
        lnk_s = np.tile(kn_w, 2).reshape(128, 1).astype(np.float32)
        lnk_b = np.tile(kn_b, 2).reshape(128, 1).astype(np.float32)

    in_maps = []
    for c in range(NCORES):
        b, g = divmod(c, 4)
        hs = slice(g * G * D, (g + 1) * G * D)          # 256 cols of this group
        w_loc = np.concatenate([qkv_w[0 * C:1 * C][hs],
                                qkv_w[1 * C:2 * C][hs],
                                qkv_w[2 * C:3 * C][hs]], axis=0)   # (768, 1024)
        b_loc = np.concatenate([qkv_b[0 * C:1 * C][hs],
                                qkv_b[1 * C:2 * C][hs],
                                qkv_b[2 * C:3 * C][hs]])[None, :]  # (1, 768)
        in_maps.append({
            "xt": np.ascontiguousarray(
                x[b].reshape(TT, 128, KI, 128).transpose(3, 0, 2, 1)).astype(mmnp),
            "wqkv_t": np.ascontiguousarray(w_loc.T).astype(mmnp),
            "bqkv": np.ascontiguousarray(b_loc).astype(mmnp),
            "wproj_t": np.ascontiguousarray(proj_w[:, hs].T).astype(mmnp),
            "lnq_s": lnq_s, "lnq_b": lnq_b,
            "lnk_s": lnk_s, "lnk_b": lnk_b,
        })
    return in_maps, fold


def run(inputs, trace=False, dve_cap_frac=1.0, debug=False):
    """Run on hardware; returns (full_output, BassKernelResults)."""
    proj_b = np.asarray(inputs["proj_b"], np.float32)
    in_maps, fold = _make_in_maps(
        inputs["x"], inputs["qkv_w"], inputs["qkv_b"],
        inputs["qn_w"], inputs["qn_b"], inputs["kn_w"], inputs["kn_b"],
        inputs["proj_w"])
    nc = _get_nc(dve_cap_frac, debug, fold_scales=fold)
    res = run_bass_kernel_spmd(nc, in_maps, core_ids=list(range(NCORES)),
                               trace=trace)
    out = np.zeros((B, N, C), np.float32)
    for b in range(B):
        acc = res.results[b * 4 + 0]["out_fm"].copy()
        for g in range(1, 4):
            acc += res.results[b * 4 + g]["out_fm"]
        out[b] = acc.T + proj_b
    return out, res


def kernel(**inputs) -> np.ndarray:
    out, _ = run(inputs, trace=False)
    return out

